# revision 13
# baseline (speedup 1.0000x reference)
"""ViT attention block (B=8, N=1024, dim=1024, heads=16, d_k=64) on 8 trn2 NeuronCores.

Sharding: data-parallel over batch (1 batch per core), weights replicated.
No collectives needed; each core computes its batch's full attention output.

Per-core algorithm (all matmuls on TensorE contract over the partition dim):
  - host pre-transposes x[b] -> xT [dim, tokens] so QKV projections can use
    w_qkv (natural layout) as the stationary operand.
  - QT/KT = (w_qkv[:, :2048]).T @ xT  -> [2048, tokens]; head pair 2t,2t+1
    lives in partition-tile t ([128, 1024]), i.e. heads' d_k=64 rows stacked.
  - V = xT.T @ w_qkv[:, 2048:]       -> [tokens, 1024], stored with a
    constant-1 column appended per head (65 cols/head) so the PV matmul
    produces softmax row-sums for free.
  - per head pair: S^T[m,n] = (KT tile).T @ QT (K=64 contraction; the two
    heads run as concurrent row-group matmuls via tile_position).
    exp(scale*S^T) on ScalarE directly out of PSUM -> E^T bf16 in SBUF.
    (max-subtraction is skipped: |scale*S| <~ 2 here, exp is exact-safe and
    softmax is shift-invariant.)
  - PV: out^T[d'+1, n] = V'.T @ E^T accumulated over m tiles; row 64 is the
    softmax denominator. The PSUM tile is staged to SBUF with one copy
    (fast PSUM release); the denominator row is reshaped via DRAM to
    [128, 8] for a full-width reciprocal, broadcast back via a
    partition-broadcast DMA, and fused into the normalize multiply.
  - final = attnT.T @ w_out + b_out, evicted fp32 and DMA'd out.

Schedule: the per-pair loop is software-pipelined to keep TensorE dense
(HAM stays at K=8/8) while ScalarE streams exps:
  slot mt of pair p emits:  QKT filler matmuls for pair p+1,
                            S^T(p, mt) + exp(p, mt),
                            PV(p-1) chunk (h1 in slots 0-3, h2 in 4-7).
"""

import os
import numpy as np
import ml_dtypes

import concourse.bass as bass
from concourse import bacc
import concourse.mybir as mybir
import concourse.tile as tile
from concourse.bass_utils import run_bass_kernel_spmd

P = 128
N_TOK = 1024
DIM = 1024
HEADS = 16
D_K = 64
N_CORES = 8
SCALE = D_K ** -0.5  # 0.125

NP_T = N_TOK // P   # 8 token tiles
DP = DIM // P       # 8 dim tiles
NPAIRS = HEADS // 2  # 8 head pairs
VW = D_K + 1        # 65: V columns per head incl. ones column

# matmul operand dtype: "bf16" | "fp32r" | "fp32"
MM_DTYPE = os.environ.get("KERNEL_MM_DTYPE", "bf16")
_DT = {
    "bf16": mybir.dt.bfloat16,
    "fp32r": mybir.dt.float32r,
    "fp32": mybir.dt.float32,
}[MM_DTYPE]
_NPDT = {"bf16": ml_dtypes.bfloat16, "fp32r": np.float32, "fp32": np.float32}[MM_DTYPE]

F32 = mybir.dt.float32


def build_program():
    nc = bacc.Bacc("TRN2", target_bir_lowering=False, debug=False)

    xT = nc.dram_tensor("xT", [DIM, N_TOK], _DT, kind="ExternalInput").ap()
    wqkv = nc.dram_tensor("w_qkv", [DIM, 3 * DIM], _DT, kind="ExternalInput").ap()
    wout = nc.dram_tensor("w_out", [DIM, DIM], _DT, kind="ExternalInput").ap()
    bout = nc.dram_tensor("b_out", [DIM], F32, kind="ExternalInput").ap()
    out = nc.dram_tensor("out", [N_TOK, DIM], F32, kind="ExternalOutput").ap()
    # denominator bounce buffers (raw row, then reciprocal row)
    rs_dram = nc.dram_tensor("rs_scratch", [HEADS, N_TOK], F32).ap()
    rs2_dram = nc.dram_tensor("rs2_scratch", [HEADS, N_TOK], F32).ap()

    with tile.TileContext(nc) as tc:
        with (
            tc.tile_pool(name="persist", bufs=1) as persist,
            tc.tile_pool(name="qkt", bufs=6) as qktp,
            tc.tile_pool(name="wqk", bufs=24) as wqkp,
        ):
            xT_sb = []
            v_sb = []      # per token-tile: [128, 16*65]
            attnT_sb = []  # per pair: [128, 1024] = two heads' [64, n]
            for j in range(NP_T):
                v_sb.append(persist.tile([P, HEADS * VW], _DT, tag=f"v{j}",
                                         name=f"v{j}"))
            for p in range(NPAIRS):
                attnT_sb.append(persist.tile([P, N_TOK], _DT, tag=f"attnT{p}",
                                             name=f"attnT{p}"))

            def make_qkt_tile(j, pool):
                """Emit QKT M-tile j ([128, tokens] slice of QKV^T) in full."""
                ps = pool.tile([P, N_TOK], F32, tag="pq", name=f"psqk{j}")
                for k in range(DP):
                    w = wqkp.tile([P, P], _DT, tag="wqk", name=f"w{j}_{k}")
                    nc.sync.dma_start(w[:], wqkv[k * P:(k + 1) * P,
                                                 j * P:(j + 1) * P])
                    for nh in range(2):
                        nc.tensor.matmul(
                            ps[:, nh * 512:(nh + 1) * 512],
                            lhsT=w[:],
                            rhs=xT_sb[k][:, nh * 512:(nh + 1) * 512],
                            start=(k == 0), stop=(k == DP - 1),
                        )
                t = qktp.tile([P, N_TOK], _DT, tag="qkt", name=f"qkt{j}")
                nc.vector.tensor_copy(out=t[:], in_=ps[:])
                return t

            # ============ phase 1: V' and pair-0 QT/KT ============
            # V weights live in a scoped pool that frees before ET opens.
            # DMA order matters: interleave xT/wv so V matmuls start early.
            with (
                tc.tile_pool(name="wvp", bufs=1) as wvp,
                tc.tile_pool(name="pq1", bufs=2, space="PSUM") as pq1,
            ):
                wv_sb = []
                for k in range(DP):
                    t = persist.tile([P, N_TOK], _DT, tag=f"xT{k}",
                                     name=f"xT{k}")
                    nc.sync.dma_start(t[:], xT[k * P:(k + 1) * P, :])
                    xT_sb.append(t)
                    w = wvp.tile([P, DIM], _DT, tag=f"wv{k}", name=f"wv{k}")
                    nc.sync.dma_start(w[:], wqkv[k * P:(k + 1) * P, 2 * DIM:])
                    wv_sb.append(w)
                for j in range(NP_T):
                    vt = v_sb[j]
                    nc.vector.memset(
                        vt[:].rearrange("p (h x) -> p h x", x=VW)[:, :, D_K:], 1.0)
                    ps = pq1.tile([P, DIM], F32, tag="pq", name=f"psv{j}")
                    for k in range(DP):
                        for nh in range(2):
                            nc.tensor.matmul(
                                ps[:, nh * 512:(nh + 1) * 512],
                                lhsT=xT_sb[k][:, j * P:(j + 1) * P],
                                rhs=wv_sb[k][:, nh * 512:(nh + 1) * 512],
                                start=(k == 0), stop=(k == DP - 1),
                            )
                    nc.vector.tensor_copy(
                        out=vt[:].rearrange("p (h x) -> p h x", x=VW)[:, :, :D_K],
                        in_=ps[:].rearrange("p (h d) -> p h d", d=D_K),
                    )
                qt_cur = make_qkt_tile(0, pq1)
                kt_cur = make_qkt_tile(DP + 0, pq1)

            # ============ phase 2: pipelined attention ============
            with (
                tc.tile_pool(name="et", bufs=18) as etp,
                tc.tile_pool(name="stg", bufs=3) as stgp,
                tc.tile_pool(name="small", bufs=2) as small,
                tc.tile_pool(name="woutp", bufs=1) as woutp,
                tc.tile_pool(name="ppv", bufs=1, space="PSUM") as ppv,
            ):
                wout_sb = []
                for k in range(DP):
                    w = woutp.tile([P, DIM], _DT, tag=f"wout{k}",
                                   name=f"wout{k}")
                    nc.sync.dma_start(w[:], wout[k * P:(k + 1) * P, :])
                    wout_sb.append(w)
                bias_bc = persist.tile([P, DIM], F32, tag="bias")
                bias_in = bass.AP(tensor=bout.tensor, offset=bout.offset,
                                  ap=[[0, P]] + list(bout.ap))
                nc.sync.dma_start(bias_bc[:], bias_in)
                et_tiles = {}   # (pair, mt) -> ET tile [128, 2048]
                inflight = {}   # accumulation state for pv / filler psums

                def normalize_evict(p, h, pv_tile):
                    """Stage pv psum to SBUF (releases PSUM), then normalize by
                    the softmax denominator and write into attnT_sb[p]."""
                    hg = 2 * p + h
                    stg = stgp.tile([VW, N_TOK], F32, tag="stg",
                                    name=f"stg{hg}")
                    nc.vector.tensor_copy(out=stg[:], in_=pv_tile[:])
                    # denominator row -> DRAM -> [128, 8] for wide reciprocal
                    nc.sync.dma_start(rs_dram[hg:hg + 1, :], stg[D_K:VW, :])
                    rsp = small.tile([P, NP_T], F32, tag="rsp", name=f"rsp{hg}")
                    nc.sync.dma_start(
                        rsp[:], rs_dram[hg].rearrange("(p i) -> p i", p=P))
                    rspr = small.tile([P, NP_T], F32, tag="rspr",
                                      name=f"rspr{hg}")
                    nc.vector.reciprocal(rspr[:], rsp[:])
                    nc.sync.dma_start(
                        rs2_dram[hg].rearrange("(p i) -> p i", p=P), rspr[:])
                    rs_row = rs2_dram[hg:hg + 1, :]
                    rs_bc = bass.AP(tensor=rs_row.tensor, offset=rs_row.offset,
                                    ap=[[0, D_K], list(rs_row.ap)[-1]])
                    rcp = small.tile([D_K, N_TOK], F32, tag="rcp",
                                     name=f"rcp{hg}")
                    nc.sync.dma_start(rcp[:], rs_bc)
                    if h == 0:
                        nc.vector.tensor_mul(out=attnT_sb[p][0:D_K, :],
                                             in0=stg[0:D_K, :], in1=rcp[:])
                    else:
                        tmp = small.tile([D_K, N_TOK], _DT, tag="oddtmp",
                                         name=f"oddtmp{hg}")
                        nc.vector.tensor_mul(out=tmp[:],
                                             in0=stg[0:D_K, :], in1=rcp[:])
                        nc.sync.dma_start(attnT_sb[p][D_K:P, :], tmp[:])

                def pv_chunk(p, h, slot):
                    """4 PV matmuls for head h of pair p: m-tiles 2*slot and
                    2*slot+1, both n-halves. slot in 0..3."""
                    hg = 2 * p + h
                    if slot == 0:
                        inflight[(p, h)] = ppv.tile(
                            [VW, N_TOK], F32, tag="ppv", name=f"pv{p}_{h}")
                    pvt = inflight[(p, h)]
                    for mt in (2 * slot, 2 * slot + 1):
                        et = et_tiles[(p, mt)]
                        for nh in range(2):
                            nc.tensor.matmul(
                                pvt[:, nh * 512:(nh + 1) * 512],
                                lhsT=v_sb[mt][:, hg * VW:(hg + 1) * VW],
                                rhs=et[:, h * N_TOK + nh * 512:
                                       h * N_TOK + (nh + 1) * 512],
                                start=(mt == 0), stop=(mt == NP_T - 1),
                            )
                    if slot == 3:
                        normalize_evict(p, h, pvt)
                        del inflight[(p, h)]
                        if h == 1:
                            for mt in range(NP_T):
                                del et_tiles[(p, mt)]

                def filler_chunk(pnext, mt):
                    """4 QKT matmuls for pair pnext: M-tile qt (mt 0-3) or kt
                    (mt 4-7), k-values 2*(mt%4) and 2*(mt%4)+1, both n-halves.
                    Returns the finished SBUF tile after the 4th chunk."""
                    is_kt = mt >= 4
                    j = (DP + pnext) if is_kt else pnext
                    s = mt % 4
                    key = ("fill", pnext, is_kt)
                    if s == 0:
                        inflight[key] = pq2.tile([P, N_TOK], F32, tag="pq",
                                                 name=f"psf{j}")
                    ps = inflight[key]
                    for k in (2 * s, 2 * s + 1):
                        w = wqkp.tile([P, P], _DT, tag="wqk", name=f"wf{j}_{k}")
                        nc.sync.dma_start(w[:], wqkv[k * P:(k + 1) * P,
                                                     j * P:(j + 1) * P])
                        for nh in range(2):
                            nc.tensor.matmul(
                                ps[:, nh * 512:(nh + 1) * 512],
                                lhsT=w[:],
                                rhs=xT_sb[k][:, nh * 512:(nh + 1) * 512],
                                start=(k == 0), stop=(k == DP - 1),
                            )
                    if s == 3:
                        t = qktp.tile([P, N_TOK], _DT, tag="qkt",
                                      name=f"qkt{j}")
                        nc.vector.tensor_copy(out=t[:], in_=ps[:])
                        del inflight[key]
                        return t
                    return None

                with (
                    tc.tile_pool(name="pst", bufs=1, space="PSUM") as pst,
                    tc.tile_pool(name="pq2", bufs=1, space="PSUM") as pq2,
                ):
                  for p in range(NPAIRS):
                    qt_next = kt_next = None
                    for mt in range(NP_T):
                        # QKT filler for pair p+1
                        if p + 1 < NPAIRS:
                            t = filler_chunk(p + 1, mt)
                            if t is not None:
                                if mt < 4:
                                    qt_next = t
                                else:
                                    kt_next = t
                        # S^T + exp for (p, mt)
                        st = pst.tile([P, 2 * N_TOK], F32, tag="pst",
                                      name=f"st{p}_{mt}")
                        et = etp.tile([P, 2 * N_TOK], _DT, tag="et",
                                      name=f"et{p}_{mt}")
                        for h in range(2):
                            for nh in range(2):
                                nc.tensor.matmul(
                                    st[:, h * N_TOK + nh * 512:
                                       h * N_TOK + (nh + 1) * 512],
                                    lhsT=kt_cur[h * D_K:(h + 1) * D_K,
                                                mt * P:(mt + 1) * P],
                                    rhs=qt_cur[h * D_K:(h + 1) * D_K,
                                               nh * 512:(nh + 1) * 512],
                                    start=True, stop=True,
                                    tile_position=(h * D_K, 0),
                                )
                            # per-head exp: frees this head's PSUM banks for
                            # the next m-tile's S^T while the other half runs
                            nc.scalar.activation(
                                et[:, h * N_TOK:(h + 1) * N_TOK],
                                st[:, h * N_TOK:(h + 1) * N_TOK],
                                mybir.ActivationFunctionType.Exp,
                                scale=float(SCALE))
                        et_tiles[(p, mt)] = et
                        # PV chunk for pair p-1
                        if p > 0:
                            pv_chunk(p - 1, mt // 4, mt % 4)
                    qt_cur, kt_cur = qt_next, kt_next

                # ==== drain last pair's PV, overlapped with projection ====
                with (
                    tc.tile_pool(name="ev", bufs=2) as ev,
                    tc.tile_pool(name="pproj", bufs=2, space="PSUM") as pproj,
                ):
                    for h in range(2):
                        for slot in range(4):
                            pv_chunk(NPAIRS - 1, h, slot)
                    # proj: pairs 0..6 accumulate first so the pair-7 chain
                    # latency is hidden; its matmul lands last (stop=True).
                    for j in range(NP_T):
                        ps = pproj.tile([P, DIM], F32, tag="pproj",
                                        name=f"pso{j}")
                        for p in range(NPAIRS):
                            for nh in range(2):
                                nc.tensor.matmul(
                                    ps[:, nh * 512:(nh + 1) * 512],
                                    lhsT=attnT_sb[p][:, j * P:(j + 1) * P],
                                    rhs=wout_sb[p][:, nh * 512:(nh + 1) * 512],
                                    start=(p == 0), stop=(p == NPAIRS - 1),
                                )
                        o = ev.tile([P, DIM], F32, tag="out", name=f"o{j}")
                        nc.vector.tensor_add(out=o[:], in0=ps[:], in1=bias_bc[:])
                        nc.sync.dma_start(out[j * P:(j + 1) * P, :], o[:])

    nc.compile()
    return nc


_NC_CACHE = None


def _get_program():
    global _NC_CACHE
    if _NC_CACHE is None:
        _NC_CACHE = build_program()
    return _NC_CACHE


def make_in_maps(x, w_qkv, w_out, b_out):
    w_qkv_c = np.ascontiguousarray(w_qkv).astype(_NPDT)
    w_out_c = np.ascontiguousarray(w_out).astype(_NPDT)
    b_out_c = np.ascontiguousarray(b_out).astype(np.float32)
    in_maps = []
    for b in range(N_CORES):
        xTb = np.ascontiguousarray(np.asarray(x[b]).T).astype(_NPDT)
        in_maps.append({
            "xT": xTb,
            "w_qkv": w_qkv_c,
            "w_out": w_out_c,
            "b_out": b_out_c,
        })
    return in_maps


def kernel(x, w_qkv, w_out, b_out):
    nc = _get_program()
    in_maps = make_in_maps(x, w_qkv, w_out, b_out)
    res = run_bass_kernel_spmd(nc, in_maps, list(range(N_CORES)))
    outs = [np.asarray(r["out"], dtype=np.float32) for r in res.results]
    return np.stack(outs, axis=0)


# revision 14
# speedup vs baseline: 1.2392x; 1.2392x over previous
"""ViT attention block (B=8, N=1024, dim=1024, heads=16, d_k=64) on 8 trn2 NeuronCores.

Sharding: data-parallel over batch (1 batch per core), weights replicated.
No collectives needed; each core computes its batch's full attention output.

Per-core algorithm (all matmuls on TensorE contract over the partition dim):
  - host pre-transposes x[b] -> xT [dim, tokens] so QKV projections can use
    w_qkv (natural layout) as the stationary operand.
  - QT/KT = (w_qkv[:, :2048]).T @ xT  -> [2048, tokens]; head pair 2t,2t+1
    lives in partition-tile t ([128, 1024]), i.e. heads' d_k=64 rows stacked.
  - V = xT.T @ w_qkv[:, 2048:]       -> [tokens, 1024], stored with a
    constant-1 column appended per head (65 cols/head) so the PV matmul
    produces softmax row-sums for free.
  - per head pair: S^T[m,n] = (KT tile).T @ QT (K=64 contraction; the two
    heads run as concurrent row-group matmuls via tile_position).
    exp(scale*S^T) on ScalarE directly out of PSUM -> E^T bf16 in SBUF.
    (max-subtraction is skipped: |scale*S| <~ 2 here, exp is exact-safe and
    softmax is shift-invariant.)
  - PV: out^T[d'+1, n] = V'.T @ E^T accumulated over m tiles; row 64 is the
    softmax denominator. The PSUM tile is staged to SBUF with one copy
    (fast PSUM release); the denominator row is reshaped via DRAM to
    [128, 8] for a full-width reciprocal, broadcast back via a
    partition-broadcast DMA, and fused into the normalize multiply.
  - final = attnT.T @ w_out + b_out, evicted fp32 and DMA'd out.

Schedule: the per-pair loop is software-pipelined to keep TensorE dense
(HAM stays at K=8/8) while ScalarE streams exps:
  slot mt of pair p emits:  QKT filler matmuls for pair p+1,
                            S^T(p, mt) + exp(p, mt),
                            PV(p-1) chunk (h1 in slots 0-3, h2 in 4-7).
"""

import os
import numpy as np
import ml_dtypes

import concourse.bass as bass
from concourse import bacc
import concourse.mybir as mybir
import concourse.tile as tile
from concourse.bass_utils import run_bass_kernel_spmd

P = 128
N_TOK = 1024
DIM = 1024
HEADS = 16
D_K = 64
N_CORES = 8
SCALE = D_K ** -0.5  # 0.125

NP_T = N_TOK // P   # 8 token tiles
DP = DIM // P       # 8 dim tiles
NPAIRS = HEADS // 2  # 8 head pairs
VW = D_K + 1        # 65: V columns per head incl. ones column

# matmul operand dtype: "bf16" | "fp32r" | "fp32"
MM_DTYPE = os.environ.get("KERNEL_MM_DTYPE", "bf16")
_DT = {
    "bf16": mybir.dt.bfloat16,
    "fp32r": mybir.dt.float32r,
    "fp32": mybir.dt.float32,
}[MM_DTYPE]
_NPDT = {"bf16": ml_dtypes.bfloat16, "fp32r": np.float32, "fp32": np.float32}[MM_DTYPE]

F32 = mybir.dt.float32


def build_program():
    nc = bacc.Bacc("TRN2", target_bir_lowering=False, debug=False)

    xT = nc.dram_tensor("xT", [DIM, N_TOK], _DT, kind="ExternalInput").ap()
    wqkv = nc.dram_tensor("w_qkv", [DIM, 3 * DIM], _DT, kind="ExternalInput").ap()
    wout = nc.dram_tensor("w_out", [DIM, DIM], _DT, kind="ExternalInput").ap()
    bout = nc.dram_tensor("b_out", [DIM], F32, kind="ExternalInput").ap()
    out = nc.dram_tensor("out", [N_TOK, DIM], F32, kind="ExternalOutput").ap()
    # denominator bounce buffers (raw row, then reciprocal row)
    rs_dram = nc.dram_tensor("rs_scratch", [HEADS, N_TOK], F32).ap()
    rs2_dram = nc.dram_tensor("rs2_scratch", [HEADS, N_TOK], F32).ap()

    with tile.TileContext(nc) as tc:
        with (
            tc.tile_pool(name="persist", bufs=1) as persist,
            tc.tile_pool(name="qkt", bufs=6) as qktp,
            tc.tile_pool(name="wqk", bufs=24) as wqkp,
        ):
            xT_sb = []
            v_sb = []      # per token-tile: [128, 16*65]
            attnT_sb = []  # per pair: [128, 1024] = two heads' [64, n]
            for j in range(NP_T):
                v_sb.append(persist.tile([P, HEADS * VW], _DT, tag=f"v{j}",
                                         name=f"v{j}"))
            for p in range(NPAIRS):
                attnT_sb.append(persist.tile([P, N_TOK], _DT, tag=f"attnT{p}",
                                             name=f"attnT{p}"))

            def make_qkt_tile(j, pool):
                """Emit QKT M-tile j ([128, tokens] slice of QKV^T) in full."""
                ps = pool.tile([P, N_TOK], F32, tag="pq", name=f"psqk{j}")
                for k in range(DP):
                    w = wqkp.tile([P, P], _DT, tag="wqk", name=f"w{j}_{k}")
                    nc.sync.dma_start(w[:], wqkv[k * P:(k + 1) * P,
                                                 j * P:(j + 1) * P])
                    for nh in range(2):
                        nc.tensor.matmul(
                            ps[:, nh * 512:(nh + 1) * 512],
                            lhsT=w[:],
                            rhs=xT_sb[k][:, nh * 512:(nh + 1) * 512],
                            start=(k == 0), stop=(k == DP - 1),
                        )
                t = qktp.tile([P, N_TOK], _DT, tag="qkt", name=f"qkt{j}")
                nc.vector.tensor_copy(out=t[:], in_=ps[:])
                return t

            # ============ phase 1: V' and pair-0 QT/KT ============
            # V weights live in a scoped pool that frees before ET opens.
            # DMA order matters: interleave xT/wv so V matmuls start early.
            with (
                tc.tile_pool(name="wvp", bufs=1) as wvp,
                tc.tile_pool(name="pq1", bufs=2, space="PSUM") as pq1,
            ):
                wv_sb = []
                for k in range(DP):
                    t = persist.tile([P, N_TOK], _DT, tag=f"xT{k}",
                                     name=f"xT{k}")
                    nc.sync.dma_start(t[:], xT[k * P:(k + 1) * P, :])
                    xT_sb.append(t)
                    w = wvp.tile([P, DIM], _DT, tag=f"wv{k}", name=f"wv{k}")
                    nc.sync.dma_start(w[:], wqkv[k * P:(k + 1) * P, 2 * DIM:])
                    wv_sb.append(w)
                for j in range(NP_T):
                    vt = v_sb[j]
                    nc.vector.memset(
                        vt[:].rearrange("p (h x) -> p h x", x=VW)[:, :, D_K:], 1.0)
                    ps = pq1.tile([P, DIM], F32, tag="pq", name=f"psv{j}")
                    for k in range(DP):
                        for nh in range(2):
                            nc.tensor.matmul(
                                ps[:, nh * 512:(nh + 1) * 512],
                                lhsT=xT_sb[k][:, j * P:(j + 1) * P],
                                rhs=wv_sb[k][:, nh * 512:(nh + 1) * 512],
                                start=(k == 0), stop=(k == DP - 1),
                            )
                    nc.vector.tensor_copy(
                        out=vt[:].rearrange("p (h x) -> p h x", x=VW)[:, :, :D_K],
                        in_=ps[:].rearrange("p (h d) -> p h d", d=D_K),
                    )
                qt_cur = make_qkt_tile(0, pq1)
                kt_cur = make_qkt_tile(DP + 0, pq1)

            # ============ phase 2: pipelined attention ============
            with (
                tc.tile_pool(name="et", bufs=18) as etp,
                tc.tile_pool(name="stg", bufs=3) as stgp,
                tc.tile_pool(name="small", bufs=2) as small,
                tc.tile_pool(name="woutp", bufs=1) as woutp,
                tc.tile_pool(name="ppv", bufs=2, space="PSUM") as ppv,
            ):
                wout_sb = []
                for k in range(DP):
                    w = woutp.tile([P, DIM], _DT, tag=f"wout{k}",
                                   name=f"wout{k}")
                    nc.sync.dma_start(w[:], wout[k * P:(k + 1) * P, :])
                    wout_sb.append(w)
                bias_bc = persist.tile([P, DIM], F32, tag="bias")
                bias_in = bass.AP(tensor=bout.tensor, offset=bout.offset,
                                  ap=[[0, P]] + list(bout.ap))
                nc.sync.dma_start(bias_bc[:], bias_in)
                et_tiles = {}   # (pair, mt) -> ET tile [128, 2048]
                inflight = {}   # accumulation state for pv / filler psums

                def normalize_evict(p, h, stg):
                    """Normalize the staged PV result by the softmax
                    denominator (row 64) and write into attnT_sb[p]."""
                    hg = 2 * p + h
                    # denominator row -> DRAM -> [128, 8] for wide reciprocal
                    nc.sync.dma_start(rs_dram[hg:hg + 1, :], stg[D_K:VW, :])
                    rsp = small.tile([P, NP_T], F32, tag="rsp", name=f"rsp{hg}")
                    nc.sync.dma_start(
                        rsp[:], rs_dram[hg].rearrange("(p i) -> p i", p=P))
                    rspr = small.tile([P, NP_T], F32, tag="rspr",
                                      name=f"rspr{hg}")
                    nc.vector.reciprocal(rspr[:], rsp[:])
                    nc.sync.dma_start(
                        rs2_dram[hg].rearrange("(p i) -> p i", p=P), rspr[:])
                    rs_row = rs2_dram[hg:hg + 1, :]
                    rs_bc = bass.AP(tensor=rs_row.tensor, offset=rs_row.offset,
                                    ap=[[0, D_K], list(rs_row.ap)[-1]])
                    rcp = small.tile([D_K, N_TOK], F32, tag="rcp",
                                     name=f"rcp{hg}")
                    nc.sync.dma_start(rcp[:], rs_bc)
                    if h == 0:
                        nc.vector.tensor_mul(out=attnT_sb[p][0:D_K, :],
                                             in0=stg[0:D_K, :], in1=rcp[:])
                    else:
                        tmp = small.tile([D_K, N_TOK], _DT, tag="oddtmp",
                                         name=f"oddtmp{hg}")
                        nc.vector.tensor_mul(out=tmp[:],
                                             in0=stg[0:D_K, :], in1=rcp[:])
                        nc.sync.dma_start(attnT_sb[p][D_K:P, :], tmp[:])

                def pv_chunk(p, slot8):
                    """4 PV matmuls for pair p. Passes of 8 MMs: (h, nh) =
                    slot8//2, each pass covers all m-tiles in 2 slots using a
                    1-bank psum tile; evicted into the pvstage half."""
                    h, nh = slot8 // 4, (slot8 // 2) % 2
                    hg = 2 * p + h
                    half = slot8 % 2  # first or second 4 m-tiles
                    if half == 0:
                        inflight[(p, h, nh)] = ppv.tile(
                            [VW, 512], F32, tag="ppv", name=f"pv{p}_{h}_{nh}")
                    pvt = inflight[(p, h, nh)]
                    for mt in range(4 * half, 4 * half + 4):
                        et = et_tiles[(p, mt)]
                        nc.tensor.matmul(
                            pvt[:],
                            lhsT=v_sb[mt][:, hg * VW:(hg + 1) * VW],
                            rhs=et[:, h * N_TOK + nh * 512:
                                   h * N_TOK + (nh + 1) * 512],
                            start=(mt == 0), stop=(mt == NP_T - 1),
                        )
                    if half == 1:
                        if nh == 0:
                            inflight[("stg", p, h)] = stgp.tile(
                                [VW, N_TOK], F32, tag="stg", name=f"stg{hg}")
                        stg = inflight[("stg", p, h)]
                        nc.vector.tensor_copy(
                            out=stg[:, nh * 512:(nh + 1) * 512], in_=pvt[:])
                        del inflight[(p, h, nh)]
                        if nh == 1:
                            normalize_evict(p, h, stg)
                            del inflight[("stg", p, h)]
                            if h == 1:
                                for mt in range(NP_T):
                                    del et_tiles[(p, mt)]

                def filler_chunk(pnext, mt):
                    """4 QKT matmuls for pair pnext: M-tile qt (mt 0-3) or kt
                    (mt 4-7), k-values 2*(mt%4) and 2*(mt%4)+1, both n-halves.
                    Returns the finished SBUF tile after the 4th chunk."""
                    is_kt = mt >= 4
                    j = (DP + pnext) if is_kt else pnext
                    s = mt % 4
                    key = ("fill", pnext, is_kt)
                    if s == 0:
                        inflight[key] = pq2.tile([P, N_TOK], F32, tag="pq",
                                                 name=f"psf{j}")
                    ps = inflight[key]
                    for k in (2 * s, 2 * s + 1):
                        w = wqkp.tile([P, P], _DT, tag="wqk", name=f"wf{j}_{k}")
                        nc.sync.dma_start(w[:], wqkv[k * P:(k + 1) * P,
                                                     j * P:(j + 1) * P])
                        for nh in range(2):
                            nc.tensor.matmul(
                                ps[:, nh * 512:(nh + 1) * 512],
                                lhsT=w[:],
                                rhs=xT_sb[k][:, nh * 512:(nh + 1) * 512],
                                start=(k == 0), stop=(k == DP - 1),
                            )
                    if s == 3:
                        t = qktp.tile([P, N_TOK], _DT, tag="qkt",
                                      name=f"qkt{j}")
                        nc.vector.tensor_copy(out=t[:], in_=ps[:])
                        del inflight[key]
                        return t
                    return None

                with (
                    tc.tile_pool(name="pst", bufs=1, space="PSUM") as pst,
                    tc.tile_pool(name="pq2", bufs=1, space="PSUM") as pq2,
                ):
                  for p in range(NPAIRS):
                    qt_next = kt_next = None
                    for mt in range(NP_T):
                        # QKT filler for pair p+1
                        if p + 1 < NPAIRS:
                            t = filler_chunk(p + 1, mt)
                            if t is not None:
                                if mt < 4:
                                    qt_next = t
                                else:
                                    kt_next = t
                        # S^T + exp for (p, mt)
                        st = pst.tile([P, 2 * N_TOK], F32, tag="pst",
                                      name=f"st{p}_{mt}")
                        et = etp.tile([P, 2 * N_TOK], _DT, tag="et",
                                      name=f"et{p}_{mt}")
                        for h in range(2):
                            for nh in range(2):
                                nc.tensor.matmul(
                                    st[:, h * N_TOK + nh * 512:
                                       h * N_TOK + (nh + 1) * 512],
                                    lhsT=kt_cur[h * D_K:(h + 1) * D_K,
                                                mt * P:(mt + 1) * P],
                                    rhs=qt_cur[h * D_K:(h + 1) * D_K,
                                               nh * 512:(nh + 1) * 512],
                                    start=True, stop=True,
                                    tile_position=(h * D_K, 0),
                                )
                        nc.scalar.activation(et[:], st[:],
                                             mybir.ActivationFunctionType.Exp,
                                             scale=float(SCALE))
                        et_tiles[(p, mt)] = et
                        # PV chunk for pair p-1
                        if p > 0:
                            pv_chunk(p - 1, mt)
                    qt_cur, kt_cur = qt_next, kt_next

                # ==== drain last pair's PV, overlapped with projection ====
                with (
                    tc.tile_pool(name="ev", bufs=2) as ev,
                    tc.tile_pool(name="pproj", bufs=2, space="PSUM") as pproj,
                ):
                    for slot8 in range(8):
                        pv_chunk(NPAIRS - 1, slot8)
                    # proj: pairs 0..6 accumulate first so the pair-7 chain
                    # latency is hidden; its matmul lands last (stop=True).
                    for j in range(NP_T):
                        ps = pproj.tile([P, DIM], F32, tag="pproj",
                                        name=f"pso{j}")
                        for p in range(NPAIRS):
                            for nh in range(2):
                                nc.tensor.matmul(
                                    ps[:, nh * 512:(nh + 1) * 512],
                                    lhsT=attnT_sb[p][:, j * P:(j + 1) * P],
                                    rhs=wout_sb[p][:, nh * 512:(nh + 1) * 512],
                                    start=(p == 0), stop=(p == NPAIRS - 1),
                                )
                        o = ev.tile([P, DIM], F32, tag="out", name=f"o{j}")
                        nc.vector.tensor_add(out=o[:], in0=ps[:], in1=bias_bc[:])
                        nc.sync.dma_start(out[j * P:(j + 1) * P, :], o[:])

    nc.compile()
    return nc


_NC_CACHE = None


def _get_program():
    global _NC_CACHE
    if _NC_CACHE is None:
        _NC_CACHE = build_program()
    return _NC_CACHE


def make_in_maps(x, w_qkv, w_out, b_out):
    w_qkv_c = np.ascontiguousarray(w_qkv).astype(_NPDT)
    w_out_c = np.ascontiguousarray(w_out).astype(_NPDT)
    b_out_c = np.ascontiguousarray(b_out).astype(np.float32)
    in_maps = []
    for b in range(N_CORES):
        xTb = np.ascontiguousarray(np.asarray(x[b]).T).astype(_NPDT)
        in_maps.append({
            "xT": xTb,
            "w_qkv": w_qkv_c,
            "w_out": w_out_c,
            "b_out": b_out_c,
        })
    return in_maps


def kernel(x, w_qkv, w_out, b_out):
    nc = _get_program()
    in_maps = make_in_maps(x, w_qkv, w_out, b_out)
    res = run_bass_kernel_spmd(nc, in_maps, list(range(N_CORES)))
    outs = [np.asarray(r["out"], dtype=np.float32) for r in res.results]
    return np.stack(outs, axis=0)


# revision 15
# speedup vs baseline: 1.2479x; 1.0070x over previous
"""ViT attention block (B=8, N=1024, dim=1024, heads=16, d_k=64) on 8 trn2 NeuronCores.

Sharding: data-parallel over batch (1 batch per core), weights replicated.
No collectives needed; each core computes its batch's full attention output.

Per-core algorithm (all matmuls on TensorE contract over the partition dim):
  - host pre-transposes x[b] -> xT [dim, tokens] so QKV projections can use
    w_qkv (natural layout) as the stationary operand.
  - QT/KT = (w_qkv[:, :2048]).T @ xT  -> [2048, tokens]; head pair 2t,2t+1
    lives in partition-tile t ([128, 1024]), i.e. heads' d_k=64 rows stacked.
  - V = xT.T @ w_qkv[:, 2048:]       -> [tokens, 1024], stored with a
    constant-1 column appended per head (65 cols/head) so the PV matmul
    produces softmax row-sums for free.
  - per head pair: S^T[m,n] = (KT tile).T @ QT (K=64 contraction; the two
    heads run as concurrent row-group matmuls via tile_position).
    exp(scale*S^T) on ScalarE directly out of PSUM -> E^T bf16 in SBUF.
    (max-subtraction is skipped: |scale*S| <~ 2 here, exp is exact-safe and
    softmax is shift-invariant.)
  - PV: out^T[d'+1, n] = V'.T @ E^T accumulated over m tiles; row 64 is the
    softmax denominator. The PSUM tile is staged to SBUF with one copy
    (fast PSUM release); the denominator row is reshaped via DRAM to
    [128, 8] for a full-width reciprocal, broadcast back via a
    partition-broadcast DMA, and fused into the normalize multiply.
  - final = attnT.T @ w_out + b_out, evicted fp32 and DMA'd out.

Schedule: the per-pair loop is software-pipelined to keep TensorE dense
(HAM stays at K=8/8) while ScalarE streams exps:
  slot mt of pair p emits:  QKT filler matmuls for pair p+1,
                            S^T(p, mt) + exp(p, mt),
                            PV(p-1) chunk (h1 in slots 0-3, h2 in 4-7).
"""

import os
import numpy as np
import ml_dtypes

import concourse.bass as bass
from concourse import bacc
import concourse.mybir as mybir
import concourse.tile as tile
from concourse.bass_utils import run_bass_kernel_spmd

P = 128
N_TOK = 1024
DIM = 1024
HEADS = 16
D_K = 64
N_CORES = 8
SCALE = D_K ** -0.5  # 0.125

NP_T = N_TOK // P   # 8 token tiles
DP = DIM // P       # 8 dim tiles
NPAIRS = HEADS // 2  # 8 head pairs
VW = D_K + 1        # 65: V columns per head incl. ones column

# matmul operand dtype: "bf16" | "fp32r" | "fp32"
MM_DTYPE = os.environ.get("KERNEL_MM_DTYPE", "bf16")
_DT = {
    "bf16": mybir.dt.bfloat16,
    "fp32r": mybir.dt.float32r,
    "fp32": mybir.dt.float32,
}[MM_DTYPE]
_NPDT = {"bf16": ml_dtypes.bfloat16, "fp32r": np.float32, "fp32": np.float32}[MM_DTYPE]

F32 = mybir.dt.float32


def build_program():
    nc = bacc.Bacc("TRN2", target_bir_lowering=False, debug=False)

    xT = nc.dram_tensor("xT", [DIM, N_TOK], _DT, kind="ExternalInput").ap()
    wqkv = nc.dram_tensor("w_qkv", [DIM, 3 * DIM], _DT, kind="ExternalInput").ap()
    wout = nc.dram_tensor("w_out", [DIM, DIM], _DT, kind="ExternalInput").ap()
    bout = nc.dram_tensor("b_out", [DIM], F32, kind="ExternalInput").ap()
    out = nc.dram_tensor("out", [N_TOK, DIM], F32, kind="ExternalOutput").ap()
    # denominator bounce buffers (raw row, then reciprocal row)
    rs_dram = nc.dram_tensor("rs_scratch", [HEADS, N_TOK], F32).ap()
    rs2_dram = nc.dram_tensor("rs2_scratch", [HEADS, N_TOK], F32).ap()

    with tile.TileContext(nc) as tc:
        with (
            tc.tile_pool(name="persist", bufs=1) as persist,
            tc.tile_pool(name="qkt", bufs=6) as qktp,
            tc.tile_pool(name="wqk", bufs=24) as wqkp,
        ):
            xT_sb = []
            v_sb = []      # per token-tile: [128, 16*65]
            attnT_sb = []  # per pair: [128, 1024] = two heads' [64, n]
            for j in range(NP_T):
                v_sb.append(persist.tile([P, HEADS * VW], _DT, tag=f"v{j}",
                                         name=f"v{j}"))
            for p in range(NPAIRS):
                attnT_sb.append(persist.tile([P, N_TOK], _DT, tag=f"attnT{p}",
                                             name=f"attnT{p}"))

            def make_qkt_tile(j, pool):
                """Emit QKT M-tile j ([128, tokens] slice of QKV^T) in full."""
                ps = pool.tile([P, N_TOK], F32, tag="pq", name=f"psqk{j}")
                for k in range(DP):
                    w = wqkp.tile([P, P], _DT, tag="wqk", name=f"w{j}_{k}")
                    nc.sync.dma_start(w[:], wqkv[k * P:(k + 1) * P,
                                                 j * P:(j + 1) * P])
                    for nh in range(2):
                        nc.tensor.matmul(
                            ps[:, nh * 512:(nh + 1) * 512],
                            lhsT=w[:],
                            rhs=xT_sb[k][:, nh * 512:(nh + 1) * 512],
                            start=(k == 0), stop=(k == DP - 1),
                        )
                t = qktp.tile([P, N_TOK], _DT, tag="qkt", name=f"qkt{j}")
                nc.vector.tensor_copy(out=t[:], in_=ps[:])
                return t

            # ============ phase 1: V' and pair-0 QT/KT ============
            # V weights live in a scoped pool that frees before ET opens.
            # DMA order matters: interleave xT/wv so V matmuls start early.
            with (
                tc.tile_pool(name="wvp", bufs=1) as wvp,
                tc.tile_pool(name="pq1", bufs=2, space="PSUM") as pq1,
            ):
                wv_sb = []
                for k in range(DP):
                    t = persist.tile([P, N_TOK], _DT, tag=f"xT{k}",
                                     name=f"xT{k}")
                    nc.sync.dma_start(t[:], xT[k * P:(k + 1) * P, :])
                    xT_sb.append(t)
                    w = wvp.tile([P, DIM], _DT, tag=f"wv{k}", name=f"wv{k}")
                    nc.sync.dma_start(w[:], wqkv[k * P:(k + 1) * P, 2 * DIM:])
                    wv_sb.append(w)
                for j in range(NP_T):
                    vt = v_sb[j]
                    nc.vector.memset(
                        vt[:].rearrange("p (h x) -> p h x", x=VW)[:, :, D_K:], 1.0)
                    ps = pq1.tile([P, DIM], F32, tag="pq", name=f"psv{j}")
                    for k in range(DP):
                        for nh in range(2):
                            nc.tensor.matmul(
                                ps[:, nh * 512:(nh + 1) * 512],
                                lhsT=xT_sb[k][:, j * P:(j + 1) * P],
                                rhs=wv_sb[k][:, nh * 512:(nh + 1) * 512],
                                start=(k == 0), stop=(k == DP - 1),
                            )
                    nc.vector.tensor_copy(
                        out=vt[:].rearrange("p (h x) -> p h x", x=VW)[:, :, :D_K],
                        in_=ps[:].rearrange("p (h d) -> p h d", d=D_K),
                    )
                qt_cur = make_qkt_tile(0, pq1)
                kt_cur = make_qkt_tile(DP + 0, pq1)

            # ============ phase 2: pipelined attention ============
            with (
                tc.tile_pool(name="et", bufs=18) as etp,
                tc.tile_pool(name="stg", bufs=3) as stgp,
                tc.tile_pool(name="small", bufs=2) as small,
                tc.tile_pool(name="woutp", bufs=1) as woutp,
                tc.tile_pool(name="ppv", bufs=2, space="PSUM") as ppv,
            ):
                wout_sb = []
                for k in range(DP):
                    w = woutp.tile([P, DIM], _DT, tag=f"wout{k}",
                                   name=f"wout{k}")
                    nc.sync.dma_start(w[:], wout[k * P:(k + 1) * P, :])
                    wout_sb.append(w)
                bias_bc = persist.tile([P, DIM], F32, tag="bias")
                bias_in = bass.AP(tensor=bout.tensor, offset=bout.offset,
                                  ap=[[0, P]] + list(bout.ap))
                nc.sync.dma_start(bias_bc[:], bias_in)
                et_tiles = {}   # (pair, mt) -> ET tile [128, 2048]
                inflight = {}   # accumulation state for pv / filler psums

                def normalize_evict(p, h, stg):
                    """Normalize the staged PV result by the softmax
                    denominator (row 64) and write into attnT_sb[p]."""
                    hg = 2 * p + h
                    # denominator row -> DRAM -> [128, 8] for wide
                    # reciprocal. These are latency chains, not bandwidth:
                    # keep them off the sync queue (head-of-line blocking of
                    # the streamed weight loads) by using gpsimd SWDGE.
                    nc.gpsimd.dma_start(rs_dram[hg:hg + 1, :], stg[D_K:VW, :])
                    rsp = small.tile([P, NP_T], F32, tag="rsp", name=f"rsp{hg}")
                    nc.gpsimd.dma_start(
                        rsp[:], rs_dram[hg].rearrange("(p i) -> p i", p=P))
                    rspr = small.tile([P, NP_T], F32, tag="rspr",
                                      name=f"rspr{hg}")
                    nc.vector.reciprocal(rspr[:], rsp[:])
                    nc.gpsimd.dma_start(
                        rs2_dram[hg].rearrange("(p i) -> p i", p=P), rspr[:])
                    rs_row = rs2_dram[hg:hg + 1, :]
                    rs_bc = bass.AP(tensor=rs_row.tensor, offset=rs_row.offset,
                                    ap=[[0, D_K], list(rs_row.ap)[-1]])
                    rcp = small.tile([D_K, N_TOK], F32, tag="rcp",
                                     name=f"rcp{hg}")
                    nc.gpsimd.dma_start(rcp[:], rs_bc)
                    if h == 0:
                        nc.vector.tensor_mul(out=attnT_sb[p][0:D_K, :],
                                             in0=stg[0:D_K, :], in1=rcp[:])
                    else:
                        tmp = small.tile([D_K, N_TOK], _DT, tag="oddtmp",
                                         name=f"oddtmp{hg}")
                        nc.vector.tensor_mul(out=tmp[:],
                                             in0=stg[0:D_K, :], in1=rcp[:])
                        nc.gpsimd.dma_start(attnT_sb[p][D_K:P, :], tmp[:])

                def pv_chunk(p, slot8):
                    """4 PV matmuls for pair p. Passes of 8 MMs: (h, nh) =
                    slot8//2, each pass covers all m-tiles in 2 slots using a
                    1-bank psum tile; evicted into the pvstage half."""
                    h, nh = slot8 // 4, (slot8 // 2) % 2
                    hg = 2 * p + h
                    half = slot8 % 2  # first or second 4 m-tiles
                    if half == 0:
                        inflight[(p, h, nh)] = ppv.tile(
                            [VW, 512], F32, tag="ppv", name=f"pv{p}_{h}_{nh}")
                    pvt = inflight[(p, h, nh)]
                    for mt in range(4 * half, 4 * half + 4):
                        et = et_tiles[(p, mt)]
                        nc.tensor.matmul(
                            pvt[:],
                            lhsT=v_sb[mt][:, hg * VW:(hg + 1) * VW],
                            rhs=et[:, h * N_TOK + nh * 512:
                                   h * N_TOK + (nh + 1) * 512],
                            start=(mt == 0), stop=(mt == NP_T - 1),
                        )
                    if half == 1:
                        if nh == 0:
                            inflight[("stg", p, h)] = stgp.tile(
                                [VW, N_TOK], F32, tag="stg", name=f"stg{hg}")
                        stg = inflight[("stg", p, h)]
                        nc.vector.tensor_copy(
                            out=stg[:, nh * 512:(nh + 1) * 512], in_=pvt[:])
                        del inflight[(p, h, nh)]
                        if nh == 1:
                            normalize_evict(p, h, stg)
                            del inflight[("stg", p, h)]
                            if h == 1:
                                for mt in range(NP_T):
                                    del et_tiles[(p, mt)]

                def filler_chunk(pnext, mt):
                    """4 QKT matmuls for pair pnext: M-tile qt (mt 0-3) or kt
                    (mt 4-7), k-values 2*(mt%4) and 2*(mt%4)+1, both n-halves.
                    Returns the finished SBUF tile after the 4th chunk."""
                    is_kt = mt >= 4
                    j = (DP + pnext) if is_kt else pnext
                    s = mt % 4
                    key = ("fill", pnext, is_kt)
                    if s == 0:
                        inflight[key] = pq2.tile([P, N_TOK], F32, tag="pq",
                                                 name=f"psf{j}")
                    ps = inflight[key]
                    for k in (2 * s, 2 * s + 1):
                        w = wqkp.tile([P, P], _DT, tag="wqk", name=f"wf{j}_{k}")
                        nc.sync.dma_start(w[:], wqkv[k * P:(k + 1) * P,
                                                     j * P:(j + 1) * P])
                        for nh in range(2):
                            nc.tensor.matmul(
                                ps[:, nh * 512:(nh + 1) * 512],
                                lhsT=w[:],
                                rhs=xT_sb[k][:, nh * 512:(nh + 1) * 512],
                                start=(k == 0), stop=(k == DP - 1),
                            )
                    if s == 3:
                        t = qktp.tile([P, N_TOK], _DT, tag="qkt",
                                      name=f"qkt{j}")
                        nc.vector.tensor_copy(out=t[:], in_=ps[:])
                        del inflight[key]
                        return t
                    return None

                with (
                    tc.tile_pool(name="pst", bufs=1, space="PSUM") as pst,
                    tc.tile_pool(name="pq2", bufs=1, space="PSUM") as pq2,
                ):
                  for p in range(NPAIRS):
                    qt_next = kt_next = None
                    for mt in range(NP_T):
                        # QKT filler for pair p+1
                        if p + 1 < NPAIRS:
                            t = filler_chunk(p + 1, mt)
                            if t is not None:
                                if mt < 4:
                                    qt_next = t
                                else:
                                    kt_next = t
                        # S^T + exp for (p, mt)
                        st = pst.tile([P, 2 * N_TOK], F32, tag="pst",
                                      name=f"st{p}_{mt}")
                        et = etp.tile([P, 2 * N_TOK], _DT, tag="et",
                                      name=f"et{p}_{mt}")
                        for h in range(2):
                            for nh in range(2):
                                nc.tensor.matmul(
                                    st[:, h * N_TOK + nh * 512:
                                       h * N_TOK + (nh + 1) * 512],
                                    lhsT=kt_cur[h * D_K:(h + 1) * D_K,
                                                mt * P:(mt + 1) * P],
                                    rhs=qt_cur[h * D_K:(h + 1) * D_K,
                                               nh * 512:(nh + 1) * 512],
                                    start=True, stop=True,
                                    tile_position=(h * D_K, 0),
                                )
                        nc.scalar.activation(et[:], st[:],
                                             mybir.ActivationFunctionType.Exp,
                                             scale=float(SCALE))
                        et_tiles[(p, mt)] = et
                        # PV chunk for pair p-1
                        if p > 0:
                            pv_chunk(p - 1, mt)
                    qt_cur, kt_cur = qt_next, kt_next

                # ==== drain last pair's PV, overlapped with projection ====
                with (
                    tc.tile_pool(name="ev", bufs=2) as ev,
                    tc.tile_pool(name="pproj", bufs=2, space="PSUM") as pproj,
                ):
                    for slot8 in range(8):
                        pv_chunk(NPAIRS - 1, slot8)
                    # proj: pairs 0..6 accumulate first so the pair-7 chain
                    # latency is hidden; its matmul lands last (stop=True).
                    for j in range(NP_T):
                        ps = pproj.tile([P, DIM], F32, tag="pproj",
                                        name=f"pso{j}")
                        for p in range(NPAIRS):
                            for nh in range(2):
                                nc.tensor.matmul(
                                    ps[:, nh * 512:(nh + 1) * 512],
                                    lhsT=attnT_sb[p][:, j * P:(j + 1) * P],
                                    rhs=wout_sb[p][:, nh * 512:(nh + 1) * 512],
                                    start=(p == 0), stop=(p == NPAIRS - 1),
                                )
                        o = ev.tile([P, DIM], F32, tag="out", name=f"o{j}")
                        nc.vector.tensor_add(out=o[:], in0=ps[:], in1=bias_bc[:])
                        nc.sync.dma_start(out[j * P:(j + 1) * P, :], o[:])

    nc.compile()
    return nc


_NC_CACHE = None


def _get_program():
    global _NC_CACHE
    if _NC_CACHE is None:
        _NC_CACHE = build_program()
    return _NC_CACHE


def make_in_maps(x, w_qkv, w_out, b_out):
    w_qkv_c = np.ascontiguousarray(w_qkv).astype(_NPDT)
    w_out_c = np.ascontiguousarray(w_out).astype(_NPDT)
    b_out_c = np.ascontiguousarray(b_out).astype(np.float32)
    in_maps = []
    for b in range(N_CORES):
        xTb = np.ascontiguousarray(np.asarray(x[b]).T).astype(_NPDT)
        in_maps.append({
            "xT": xTb,
            "w_qkv": w_qkv_c,
            "w_out": w_out_c,
            "b_out": b_out_c,
        })
    return in_maps


def kernel(x, w_qkv, w_out, b_out):
    nc = _get_program()
    in_maps = make_in_maps(x, w_qkv, w_out, b_out)
    res = run_bass_kernel_spmd(nc, in_maps, list(range(N_CORES)))
    outs = [np.asarray(r["out"], dtype=np.float32) for r in res.results]
    return np.stack(outs, axis=0)
